# revision 2
# baseline (speedup 1.0000x reference)
"""Trainium2 Bass kernel for ClassAttentionTSSA (v3).

Reference computation (B=64, C=256, T=64, V=25, h=8, hd=32):
    xc = x_cls  as (B, V, C) tokens;  xp = x_patch as (B, T*V, C) tokens
    q = xc @ q_w.T ; k = xp @ k_w.T ; v = xp @ v_w.T   (per-head split hd=32)
    S = (q @ k.T) * scale * temp_h ; A = softmax(S) ; o = A @ v
    y = concat_heads(o) @ proj_w.T + proj_b  -> (B, C, 1, V)

Weight-only reassociations (exact up to fp reordering):
    S_h = xc @ G_h @ xp.T    with G_h = (q_w*scale*temp)_h.T @ k_w_h  (C x C)
    y   = sum_h (A_h @ xp) @ W_h.T + b   with W_h = proj_w[:,h] @ v_w[h,:]
so q/k/v are never materialized.

v2 changes vs v1 (this environment charges ~60us PER COMPUTE-ENGINE
INSTRUCTION, globally serialized across engines; DMA is marginally free):
  * S^T matmuls use fp8e4m3 DoubleRow (K=256 in ONE instruction):
    26 -> 13 matmuls per batch.  qk is scaled by ALPHA=128 on the host so
    fp8 sees ~unit-scale values; exp() undoes it via its scale parameter.
  * ctx is computed TRANSPOSED: ctxn[r, cin] = A^T-chunks^T @ xp^T with a
    257th ones-column appended to xp^T, so the softmax denominator Z
    appears as output column 256 of the same matmuls.  The entire v1
    Z-machinery (memset + gpsimd partition_all_reduce + 13-fold reduce +
    broadcast) collapses into 2 reciprocals + 2 per-partition
    tensor_scalar_muls per batch (r is on partitions in this layout).
  * ctxn rides free DMAs through a DRAM scratch back into the
    [cin, (kc h b q)] layout phase 3 wants.
  * exp instructions grouped 4 S^T-chunks at a time (2-bank psum tiles).
ctx/phase-3 matmuls stay bf16: fp8's ~2.4% element error there would
blow the 2e-2 gate, while fp8 on S only perturbs S by ~3% of its
sigma=0.1, i.e. ~0.3% on the attention weights.

Sharding: data-parallel over batch, 8 batches per NeuronCore, 8 cores.
"""

import math
import sys

sys.path.insert(0, "/opt/trn_rl_repo")

import numpy as np
import ml_dtypes

import concourse.bacc as bacc
import concourse.mybir as mybir
import concourse.tile as tile
from concourse import bass_utils

B, C, T, V = 64, 256, 64, 25
H, HD = 8, 32
KT = T * V            # 1600 key tokens
NCORES = 8
BLOC = B // NCORES    # 8 batches per core
R = H * V             # 200 packed (head, query) columns per batch
CK = C // 128         # 2 channel chunks
C1 = C + 1            # xp^T channels + ones column (Z in-matmul)
ALPHA = 128.0         # qk pre-scale so fp8 sees ~unit values

F32 = mybir.dt.float32
BF16 = mybir.dt.bfloat16
F8 = mybir.dt.float8e4

KT_CHUNKS = [128] * (KT // 128) + ([KT % 128] if KT % 128 else [])
NM = len(KT_CHUNKS)   # 13
NM_FULL = KT // 128   # 12
EXP_G = 8             # S^T chunks per psum tile / exp instruction

_PROG_CACHE = {}


def _build_program(nreps: int = 1):
    """Build + compile the per-core Bass program (same program on all cores)."""
    from contextlib import ExitStack

    nc = bacc.Bacc("TRN2", target_bir_lowering=False, debug=False)

    xc_d = nc.dram_tensor("xc", [BLOC, C, V], F32, kind="ExternalInput")
    xpf_d = nc.dram_tensor("xpf", [BLOC, C, KT], F8, kind="ExternalInput")
    xpt_d = nc.dram_tensor("xpt", [BLOC, KT, C1], BF16, kind="ExternalInput")
    g_d = nc.dram_tensor("g", [H, C, C], BF16, kind="ExternalInput")
    w_d = nc.dram_tensor("w", [H, C, C], BF16, kind="ExternalInput")
    pb_d = nc.dram_tensor("pb", [C, 1], F32, kind="ExternalInput")
    ctx_d = nc.dram_tensor("ctxs", [BLOC, R, C], BF16, kind="Internal")
    y_d = nc.dram_tensor("y", [BLOC, C, V], F32, kind="ExternalOutput")

    with tile.TileContext(nc) as tc, ExitStack() as es:
        wpool = es.enter_context(tc.tile_pool(name="weights", bufs=1))
        qk_pool = es.enter_context(tc.tile_pool(name="qk", bufs=1))
        ctx_pool = es.enter_context(tc.tile_pool(name="ctxall", bufs=1))
        xload = es.enter_context(tc.tile_pool(name="xload", bufs=4))
        xpkt_pool = es.enter_context(tc.tile_pool(name="xpkt", bufs=4))
        attn_pool = es.enter_context(tc.tile_pool(name="attn", bufs=4))
        small_pool = es.enter_context(tc.tile_pool(name="small", bufs=4))
        ysb_pool = es.enter_context(tc.tile_pool(name="ysb", bufs=2))

        # ---- persistent weights / activations (one DMA each) ----
        g_sb = wpool.tile([128, H * CK * C], BF16, tag="g")
        nc.sync.dma_start(
            g_sb[:].rearrange("p (h kc j) -> p h kc j", h=H, kc=CK),
            g_d.ap().rearrange("h (kc p) j -> p h kc j", kc=CK),
        )
        w_sb = wpool.tile([128, H * CK * C], BF16, tag="w")
        nc.sync.dma_start(
            w_sb[:].rearrange("p (h kc j) -> p h kc j", h=H, kc=CK),
            w_d.ap().rearrange("h (kc p) j -> p h kc j", kc=CK),
        )
        pb_sb = wpool.tile([128, CK], F32, tag="pb")
        nc.sync.dma_start(
            pb_sb[:], pb_d.ap().rearrange("(kc p) one -> p (kc one)", kc=CK))
        xcT = wpool.tile([128, CK * BLOC * V], BF16, tag="xc")
        for kc in range(CK):
            nc.gpsimd.dma_start(  # SWDGE: casts f32 -> bf16 in flight
                xcT[:, kc * BLOC * V:(kc + 1) * BLOC * V].rearrange(
                    "p (b v) -> p b v", b=BLOC),
                xc_d.ap()[:, kc * 128:(kc + 1) * 128, :].rearrange(
                    "b p v -> p b v"),
            )

        # qkT cols: (kc | b, h, qi)  b-major: S^T rhs slices contiguous
        qkT = qk_pool.tile([128, CK * BLOC * R], F8, tag="qkT")
        # ctxT cols: (kc | b, h, qi) b-major so the DRAM gather merges (b,h);
        # phase-3 head slices become 3-dim strided rhs views
        ctxT = ctx_pool.tile([128, CK * BLOC * R], BF16, tag="ctxT")

        # ---- phase 1: qkT[cin, (b,h,qi)] = G_h^T @ xcT  (fp8, x ALPHA) ----
        with tc.tile_pool(name="ps_qk", bufs=2, space="PSUM") as ps_qk:
            for mc in range(CK):
                for hg in range(2):          # head groups of 4
                    pq = ps_qk.tile([128, 4 * 512], F32, tag="pq")
                    for i in range(4):
                        h = hg * 4 + i
                        for kc in range(CK):
                            nc.tensor.matmul(
                                pq[:, i * 512:i * 512 + BLOC * V],
                                g_sb[:, (h * CK + kc) * C + mc * 128:
                                     (h * CK + kc) * C + mc * 128 + 128],
                                xcT[:, kc * BLOC * V:(kc + 1) * BLOC * V],
                                start=(kc == 0), stop=(kc == CK - 1),
                            )
                    # psum cols (i | b, qi) -> qkT cols b*R + (hg*4+i)*V + qi
                    nc.vector.tensor_copy(
                        qkT[:, mc * BLOC * R:(mc + 1) * BLOC * R]
                        .rearrange("p (b h q) -> p b h q", b=BLOC, h=H)
                        [:, :, hg * 4:(hg + 1) * 4, :],
                        pq[:].rearrange("p (i n) -> p i n", i=4)
                        [:, :, 0:BLOC * V]
                        .rearrange("p i (b q) -> p b i q", q=V),
                    )

        ps_st = es.enter_context(
            tc.tile_pool(name="ps_st", bufs=1, space="PSUM"))
        ps_acc = es.enter_context(
            tc.tile_pool(name="ps_acc", bufs=2, space="PSUM"))

        qkT_v = qkT[:].rearrange("p (kc b r) -> p kc b r", kc=CK, b=BLOC)
        for _rep in range(nreps):
            # ---- phase 2: per-batch attention ----
            for b in range(BLOC):
                xpT = xload.tile([128, CK * KT], F8, tag="xpT")
                nc.sync.dma_start(
                    xpT[:].rearrange("p (kc j) -> p kc j", kc=CK),
                    xpf_d.ap()[b].rearrange("(kc p) j -> p kc j", kc=CK),
                )
                xpkt = xpkt_pool.tile([128, NM * C1], BF16, tag="xpkt")
                nc.sync.dma_start(
                    xpkt[:, 0:NM_FULL * C1].rearrange(
                        "p (m j) -> p m j", m=NM_FULL),
                    xpt_d.ap()[b, 0:NM_FULL * 128, :].rearrange(
                        "(m p) j -> p m j", p=128),
                )
                nc.sync.dma_start(
                    xpkt[0:KT - NM_FULL * 128, NM_FULL * C1:NM * C1],
                    xpt_d.ap()[b, NM_FULL * 128:KT, :],
                )
                xpT_v = xpT[:].rearrange("p (kc j) -> p kc j", kc=CK)

                # S^T chunks via fp8 DoubleRow (full K=256 per instruction),
                # EXP_G chunks per 2-bank psum tile, one exp per tile
                attn = attn_pool.tile([128, NM * R], BF16, tag="attn")
                m = 0
                while m < NM:
                    gsz = min(EXP_G, NM - m)
                    if KT_CHUNKS[m + gsz - 1] != KT_CHUNKS[m]:
                        gsz -= 1
                    rows = KT_CHUNKS[m]
                    st = ps_st.tile([128, EXP_G * 256], F32, tag="st")
                    for i in range(gsz):
                        nc.tensor.matmul(
                            st[0:KT_CHUNKS[m + i], i * 256:i * 256 + R],
                            xpT_v[:, :, (m + i) * 128:
                                  (m + i) * 128 + KT_CHUNKS[m + i]],
                            qkT_v[:, :, b, :],
                            start=True, stop=True,
                            perf_mode=mybir.MatmulPerfMode.DoubleRow,
                        )
                    nc.scalar.activation(
                        attn[0:rows, m * R:(m + gsz) * R].rearrange(
                            "p (g n) -> p g n", g=gsz),
                        st[0:rows, :].rearrange(
                            "p (g n) -> p g n", n=256)[:, 0:gsz, 0:R],
                        mybir.ActivationFunctionType.Exp,
                        scale=1.0 / ALPHA,
                    )
                    m += gsz

                # ctxn[r, cin] = sum_kt A^T[kt, r-slice]^T @ xp^T[kt, 0:257]
                # col 256 (ones) = Z; normalize with per-partition scalar.
                # two rc chunks of 100 rows share one 2-bank psum tile so a
                # single strided reciprocal covers both Z columns, and one
                # DMA stores both halves.
                RH = R // 2                            # 100
                pctx = ps_acc.tile([128, 1024], F32, tag="pctx")
                for rc in range(CK):
                    for m in range(NM):
                        nc.tensor.matmul(
                            pctx[0:RH, rc * 512:rc * 512 + C1],
                            attn[0:KT_CHUNKS[m],
                                 m * R + rc * RH:m * R + rc * RH + RH],
                            xpkt[0:KT_CHUNKS[m], m * C1:(m + 1) * C1],
                            start=(m == 0), stop=(m == NM - 1),
                        )
                rec = small_pool.tile([128, 2], F32, tag="rec")
                nc.vector.reciprocal(
                    rec[0:RH, :],
                    pctx[0:RH, :].rearrange("p (rc j) -> p rc j", rc=2)
                    [:, :, C:C1].rearrange("p rc one -> p (rc one)"))
                ctxn = small_pool.tile([128, 2 * C], BF16, tag="ctxn")
                for rc in range(CK):
                    nc.vector.tensor_scalar_mul(
                        ctxn[0:RH, rc * C:(rc + 1) * C],
                        pctx[0:RH, rc * 512:rc * 512 + C],
                        rec[0:RH, rc:rc + 1])
                nc.sync.dma_start(
                    ctx_d.ap()[b].rearrange("(rc p) c -> p rc c", rc=2),
                    ctxn[0:RH, :].rearrange("p (rc c) -> p rc c", rc=2))

            # gather ctx back as [cin, (kc b h q)] for phase 3
            for kc in range(CK):
                nc.sync.dma_start(
                    ctxT[:, kc * BLOC * R:(kc + 1) * BLOC * R].rearrange(
                        "p (b h q) -> p b h q", b=BLOC, h=H),
                    ctx_d.ap().rearrange("b (h q) (kc p) -> p kc b h q",
                                         h=H, kc=CK)[:, kc],
                )

            # ---- phase 3: y^T = sum_h W_h^T @ ctxT + pb ----
            for mc in range(CK):
                py = ps_acc.tile([128, 1024], F32, tag="pctx")
                idx = 0
                for h in range(H):
                    for kc in range(CK):
                        nc.tensor.matmul(
                            py[:, 0:BLOC * V],
                            w_sb[:, (h * CK + kc) * C + mc * 128:
                                 (h * CK + kc) * C + mc * 128 + 128],
                            ctxT[:, kc * BLOC * R:(kc + 1) * BLOC * R]
                            .rearrange("p (b h q) -> p b h q",
                                       b=BLOC, h=H)[:, :, h, :],
                            start=(idx == 0), stop=(idx == 2 * H - 1),
                        )
                        idx += 1
                ysb = ysb_pool.tile([128, BLOC * V], F32, tag="ysb")
                nc.vector.tensor_scalar_add(ysb[:], py[:, 0:BLOC * V], pb_sb[:, mc:mc + 1])
                nc.sync.dma_start(
                    y_d.ap()[:, mc * 128:(mc + 1) * 128, :].rearrange(
                        "b p v -> p b v"),
                    ysb[:].rearrange("p (b v) -> p b v", b=BLOC),
                )

    nc.compile()
    return nc


def _get_program(nreps: int = 1):
    if nreps not in _PROG_CACHE:
        _PROG_CACHE[nreps] = _build_program(nreps)
    return _PROG_CACHE[nreps]


def _host_prep(x_cls, x_patch, q_w, k_w, v_w, temp, proj_w, proj_b):
    scale = 1.0 / math.sqrt(HD)
    tvec = np.repeat(temp.reshape(H).astype(np.float64), HD)
    q_ws = q_w.astype(np.float64) * (scale * tvec)[:, None]
    k64 = k_w.astype(np.float64)
    v64 = v_w.astype(np.float64)
    p64 = proj_w.astype(np.float64)
    g = np.empty((H, C, C), dtype=np.float64)
    w = np.empty((H, C, C), dtype=np.float64)
    for h in range(H):
        sl = slice(h * HD, (h + 1) * HD)
        g[h] = q_ws[sl, :].T @ k64[sl, :]          # [cin'(K), cin(M)]
        w[h] = (p64[:, sl] @ v64[sl, :]).T         # W_h.T = [cin(K), co(M)]
    g_bf = np.ascontiguousarray((g * ALPHA).astype(ml_dtypes.bfloat16))
    w_bf = np.ascontiguousarray(w.astype(ml_dtypes.bfloat16))
    pb = np.ascontiguousarray(proj_b.reshape(C, 1).astype(np.float32))
    return g_bf, w_bf, pb


def _make_in_maps(x_cls, x_patch, g_bf, w_bf, pb):
    xp_full = x_patch.reshape(B, C, KT)
    xpf = xp_full.astype(ml_dtypes.float8_e4m3)              # [B, C, KT]
    xpt = np.empty((B, KT, C1), dtype=ml_dtypes.bfloat16)    # [B, KT, C+1]
    xpt[:, :, 0:C] = xp_full.astype(ml_dtypes.bfloat16).transpose(0, 2, 1)
    xpt[:, :, C] = np.asarray(1.0, dtype=ml_dtypes.bfloat16)
    xc = np.ascontiguousarray(x_cls.reshape(B, C, V).astype(np.float32))
    in_maps = []
    for c in range(NCORES):
        bs = slice(c * BLOC, (c + 1) * BLOC)
        in_maps.append({
            "xc": xc[bs],
            "xpf": np.ascontiguousarray(xpf[bs]),
            "xpt": np.ascontiguousarray(xpt[bs]),
            "g": g_bf, "w": w_bf, "pb": pb,
        })
    return in_maps


def kernel(x_cls, x_patch, q_w, k_w, v_w, temp, proj_w, proj_b):
    g_bf, w_bf, pb = _host_prep(
        x_cls, x_patch, q_w, k_w, v_w, temp, proj_w, proj_b)
    nc = _get_program()
    in_maps = _make_in_maps(x_cls, x_patch, g_bf, w_bf, pb)
    res = bass_utils.run_bass_kernel_spmd(
        nc, in_maps, core_ids=list(range(NCORES)))
    out = np.concatenate([res.results[c]["y"] for c in range(NCORES)], axis=0)
    return out.reshape(B, C, 1, V).astype(np.float32)


# revision 3
# speedup vs baseline: 1.5330x; 1.5330x over previous
"""Trainium2 Bass kernel for ClassAttentionTSSA (v3).

Reference computation (B=64, C=256, T=64, V=25, h=8, hd=32):
    xc = x_cls  as (B, V, C) tokens;  xp = x_patch as (B, T*V, C) tokens
    q = xc @ q_w.T ; k = xp @ k_w.T ; v = xp @ v_w.T   (per-head split hd=32)
    S = (q @ k.T) * scale * temp_h ; A = softmax(S) ; o = A @ v
    y = concat_heads(o) @ proj_w.T + proj_b  -> (B, C, 1, V)

Weight-only reassociations (exact up to fp reordering):
    S_h = xc @ G_h @ xp.T    with G_h = (q_w*scale*temp)_h.T @ k_w_h  (C x C)
    y   = sum_h (A_h @ xp) @ W_h.T + b   with W_h = proj_w[:,h] @ v_w[h,:]
so q/k/v are never materialized.

Perf model for this axon-tunneled environment (measured by microbench):
execution cost is ~60-100us PER COMPUTE-ENGINE INSTRUCTION, globally
serialized across all engines (no tensor/vector/scalar overlap), while
queued DMA instructions are marginally free.  So the only lever is
compute-instruction count.  Changes vs the v1 baseline (72ms -> 40ms
under equal load; 554 -> ~390 instructions per rep):
  * S^T matmuls use fp8e4m3 DoubleRow (2 K-tiles = K=256 in ONE
    instruction): 26 -> 13 matmuls per batch.  qk is scaled by ALPHA=128
    on the host so fp8 sees ~unit-scale values (sigma(qk)~0.006 is below
    fp8's normal range); exp() undoes it via its free scale parameter.
  * ctx is computed TRANSPOSED: ctxn[r, cin] = A^T-chunks^T @ xp^T with
    a 257th ones-column appended to xp^T, so the softmax denominator Z
    appears as output column 256 of the same matmuls.  The entire v1
    Z-machinery (memset + gpsimd partition_all_reduce @ ~1ms + 13-fold
    reduce + broadcast) collapses into 1 strided reciprocal + 2
    per-partition tensor_scalar_muls per batch (r is on partitions in
    this layout, and both 100-row r-halves share one 2-bank psum tile).
  * ctxn rides free DMAs through a DRAM scratch back into the
    [cin, (kc b h q)] b-major layout (so the gather AP merges (b,h) and
    fits the 3-dim DMA limit); phase 3 reads per-head slices as 3-dim
    strided rhs views.
  * exp grouped 8 S^T-chunks per instruction (4-bank psum tile): 2/batch.
ctx/phase-3 matmuls stay bf16: fp8's ~2.4% element error there measures
2.5-3.9% rel-to-absmax (gate is 2e-2), while fp8 on S only perturbs S by
~3% of its sigma=0.1, i.e. ~0.3% on the attention weights; total
measured error is 0.94% vs 0.34% for the all-bf16 v1.

Sharding: data-parallel over batch, 8 batches per NeuronCore, 8 cores.
"""

import math
import sys

sys.path.insert(0, "/opt/trn_rl_repo")

import numpy as np
import ml_dtypes

import concourse.bacc as bacc
import concourse.mybir as mybir
import concourse.tile as tile
from concourse import bass_utils

B, C, T, V = 64, 256, 64, 25
H, HD = 8, 32
KT = T * V            # 1600 key tokens
NCORES = 8
BLOC = B // NCORES    # 8 batches per core
R = H * V             # 200 packed (head, query) columns per batch
CK = C // 128         # 2 channel chunks
C1 = C + 1            # xp^T channels + ones column (Z in-matmul)
ALPHA = 128.0         # qk pre-scale so fp8 sees ~unit values

F32 = mybir.dt.float32
BF16 = mybir.dt.bfloat16
F8 = mybir.dt.float8e4

KT_CHUNKS = [128] * (KT // 128) + ([KT % 128] if KT % 128 else [])
NM = len(KT_CHUNKS)   # 13
NM_FULL = KT // 128   # 12
EXP_G = 8             # S^T chunks per psum tile / exp instruction

_PROG_CACHE = {}


def _build_program(nreps: int = 1):
    """Build + compile the per-core Bass program (same program on all cores)."""
    from contextlib import ExitStack

    nc = bacc.Bacc("TRN2", target_bir_lowering=False, debug=False)

    xc_d = nc.dram_tensor("xc", [BLOC, C, V], F32, kind="ExternalInput")
    xpf_d = nc.dram_tensor("xpf", [BLOC, C, KT], F8, kind="ExternalInput")
    xpt_d = nc.dram_tensor("xpt", [BLOC, KT, C1], BF16, kind="ExternalInput")
    g_d = nc.dram_tensor("g", [H, C, C], BF16, kind="ExternalInput")
    w_d = nc.dram_tensor("w", [H, C, C], BF16, kind="ExternalInput")
    pb_d = nc.dram_tensor("pb", [C, 1], F32, kind="ExternalInput")
    ctx_d = nc.dram_tensor("ctxs", [BLOC, R, C], BF16, kind="Internal")
    y_d = nc.dram_tensor("y", [BLOC, C, V], F32, kind="ExternalOutput")

    with tile.TileContext(nc) as tc, ExitStack() as es:
        wpool = es.enter_context(tc.tile_pool(name="weights", bufs=1))
        qk_pool = es.enter_context(tc.tile_pool(name="qk", bufs=1))
        ctx_pool = es.enter_context(tc.tile_pool(name="ctxall", bufs=1))
        xload = es.enter_context(tc.tile_pool(name="xload", bufs=4))
        xpkt_pool = es.enter_context(tc.tile_pool(name="xpkt", bufs=4))
        attn_pool = es.enter_context(tc.tile_pool(name="attn", bufs=4))
        small_pool = es.enter_context(tc.tile_pool(name="small", bufs=4))
        ysb_pool = es.enter_context(tc.tile_pool(name="ysb", bufs=2))

        # ---- persistent weights / activations (one DMA each) ----
        g_sb = wpool.tile([128, H * CK * C], BF16, tag="g")
        nc.sync.dma_start(
            g_sb[:].rearrange("p (h kc j) -> p h kc j", h=H, kc=CK),
            g_d.ap().rearrange("h (kc p) j -> p h kc j", kc=CK),
        )
        w_sb = wpool.tile([128, H * CK * C], BF16, tag="w")
        nc.sync.dma_start(
            w_sb[:].rearrange("p (h kc j) -> p h kc j", h=H, kc=CK),
            w_d.ap().rearrange("h (kc p) j -> p h kc j", kc=CK),
        )
        pb_sb = wpool.tile([128, CK], F32, tag="pb")
        nc.sync.dma_start(
            pb_sb[:], pb_d.ap().rearrange("(kc p) one -> p (kc one)", kc=CK))
        xcT = wpool.tile([128, CK * BLOC * V], BF16, tag="xc")
        for kc in range(CK):
            nc.gpsimd.dma_start(  # SWDGE: casts f32 -> bf16 in flight
                xcT[:, kc * BLOC * V:(kc + 1) * BLOC * V].rearrange(
                    "p (b v) -> p b v", b=BLOC),
                xc_d.ap()[:, kc * 128:(kc + 1) * 128, :].rearrange(
                    "b p v -> p b v"),
            )

        # qkT cols: (kc | b, h, qi)  b-major: S^T rhs slices contiguous
        qkT = qk_pool.tile([128, CK * BLOC * R], F8, tag="qkT")
        # ctxT cols: (kc | b, h, qi) b-major so the DRAM gather merges (b,h);
        # phase-3 head slices become 3-dim strided rhs views
        ctxT = ctx_pool.tile([128, CK * BLOC * R], BF16, tag="ctxT")

        # ---- phase 1: qkT[cin, (b,h,qi)] = G_h^T @ xcT  (fp8, x ALPHA) ----
        with tc.tile_pool(name="ps_qk", bufs=2, space="PSUM") as ps_qk:
            for mc in range(CK):
                for hg in range(2):          # head groups of 4
                    pq = ps_qk.tile([128, 4 * 512], F32, tag="pq")
                    for i in range(4):
                        h = hg * 4 + i
                        for kc in range(CK):
                            nc.tensor.matmul(
                                pq[:, i * 512:i * 512 + BLOC * V],
                                g_sb[:, (h * CK + kc) * C + mc * 128:
                                     (h * CK + kc) * C + mc * 128 + 128],
                                xcT[:, kc * BLOC * V:(kc + 1) * BLOC * V],
                                start=(kc == 0), stop=(kc == CK - 1),
                            )
                    # psum cols (i | b, qi) -> qkT cols b*R + (hg*4+i)*V + qi
                    nc.vector.tensor_copy(
                        qkT[:, mc * BLOC * R:(mc + 1) * BLOC * R]
                        .rearrange("p (b h q) -> p b h q", b=BLOC, h=H)
                        [:, :, hg * 4:(hg + 1) * 4, :],
                        pq[:].rearrange("p (i n) -> p i n", i=4)
                        [:, :, 0:BLOC * V]
                        .rearrange("p i (b q) -> p b i q", q=V),
                    )

        ps_st = es.enter_context(
            tc.tile_pool(name="ps_st", bufs=1, space="PSUM"))
        ps_acc = es.enter_context(
            tc.tile_pool(name="ps_acc", bufs=2, space="PSUM"))

        qkT_v = qkT[:].rearrange("p (kc b r) -> p kc b r", kc=CK, b=BLOC)
        for _rep in range(nreps):
            # ---- phase 2: per-batch attention ----
            for b in range(BLOC):
                xpT = xload.tile([128, CK * KT], F8, tag="xpT")
                nc.sync.dma_start(
                    xpT[:].rearrange("p (kc j) -> p kc j", kc=CK),
                    xpf_d.ap()[b].rearrange("(kc p) j -> p kc j", kc=CK),
                )
                xpkt = xpkt_pool.tile([128, NM * C1], BF16, tag="xpkt")
                nc.sync.dma_start(
                    xpkt[:, 0:NM_FULL * C1].rearrange(
                        "p (m j) -> p m j", m=NM_FULL),
                    xpt_d.ap()[b, 0:NM_FULL * 128, :].rearrange(
                        "(m p) j -> p m j", p=128),
                )
                nc.sync.dma_start(
                    xpkt[0:KT - NM_FULL * 128, NM_FULL * C1:NM * C1],
                    xpt_d.ap()[b, NM_FULL * 128:KT, :],
                )
                xpT_v = xpT[:].rearrange("p (kc j) -> p kc j", kc=CK)

                # S^T chunks via fp8 DoubleRow (full K=256 per instruction),
                # EXP_G chunks per 2-bank psum tile, one exp per tile
                attn = attn_pool.tile([128, NM * R], BF16, tag="attn")
                m = 0
                while m < NM:
                    gsz = min(EXP_G, NM - m)
                    if KT_CHUNKS[m + gsz - 1] != KT_CHUNKS[m]:
                        gsz -= 1
                    rows = KT_CHUNKS[m]
                    st = ps_st.tile([128, EXP_G * 256], F32, tag="st")
                    for i in range(gsz):
                        nc.tensor.matmul(
                            st[0:KT_CHUNKS[m + i], i * 256:i * 256 + R],
                            xpT_v[:, :, (m + i) * 128:
                                  (m + i) * 128 + KT_CHUNKS[m + i]],
                            qkT_v[:, :, b, :],
                            start=True, stop=True,
                            perf_mode=mybir.MatmulPerfMode.DoubleRow,
                        )
                    nc.scalar.activation(
                        attn[0:rows, m * R:(m + gsz) * R].rearrange(
                            "p (g n) -> p g n", g=gsz),
                        st[0:rows, :].rearrange(
                            "p (g n) -> p g n", n=256)[:, 0:gsz, 0:R],
                        mybir.ActivationFunctionType.Exp,
                        scale=1.0 / ALPHA,
                    )
                    m += gsz

                # ctxn[r, cin] = sum_kt A^T[kt, r-slice]^T @ xp^T[kt, 0:257]
                # col 256 (ones) = Z; normalize with per-partition scalar.
                # two rc chunks of 100 rows share one 2-bank psum tile so a
                # single strided reciprocal covers both Z columns, and one
                # DMA stores both halves.
                RH = R // 2                            # 100
                pctx = ps_acc.tile([128, 1024], F32, tag="pctx")
                for rc in range(CK):
                    for m in range(NM):
                        nc.tensor.matmul(
                            pctx[0:RH, rc * 512:rc * 512 + C1],
                            attn[0:KT_CHUNKS[m],
                                 m * R + rc * RH:m * R + rc * RH + RH],
                            xpkt[0:KT_CHUNKS[m], m * C1:(m + 1) * C1],
                            start=(m == 0), stop=(m == NM - 1),
                        )
                rec = small_pool.tile([128, 2], F32, tag="rec")
                nc.vector.reciprocal(
                    rec[0:RH, :],
                    pctx[0:RH, :].rearrange("p (rc j) -> p rc j", rc=2)
                    [:, :, C:C1].rearrange("p rc one -> p (rc one)"))
                ctxn = small_pool.tile([128, 2 * C], BF16, tag="ctxn")
                for rc in range(CK):
                    nc.vector.tensor_scalar_mul(
                        ctxn[0:RH, rc * C:(rc + 1) * C],
                        pctx[0:RH, rc * 512:rc * 512 + C],
                        rec[0:RH, rc:rc + 1])
                nc.sync.dma_start(
                    ctx_d.ap()[b].rearrange("(rc p) c -> p rc c", rc=2),
                    ctxn[0:RH, :].rearrange("p (rc c) -> p rc c", rc=2))

            # gather ctx back as [cin, (kc b h q)] for phase 3
            for kc in range(CK):
                nc.sync.dma_start(
                    ctxT[:, kc * BLOC * R:(kc + 1) * BLOC * R].rearrange(
                        "p (b h q) -> p b h q", b=BLOC, h=H),
                    ctx_d.ap().rearrange("b (h q) (kc p) -> p kc b h q",
                                         h=H, kc=CK)[:, kc],
                )

            # ---- phase 3: y^T = sum_h W_h^T @ ctxT + pb ----
            for mc in range(CK):
                py = ps_acc.tile([128, 1024], F32, tag="pctx")
                idx = 0
                for h in range(H):
                    for kc in range(CK):
                        nc.tensor.matmul(
                            py[:, 0:BLOC * V],
                            w_sb[:, (h * CK + kc) * C + mc * 128:
                                 (h * CK + kc) * C + mc * 128 + 128],
                            ctxT[:, kc * BLOC * R:(kc + 1) * BLOC * R]
                            .rearrange("p (b h q) -> p b h q",
                                       b=BLOC, h=H)[:, :, h, :],
                            start=(idx == 0), stop=(idx == 2 * H - 1),
                        )
                        idx += 1
                ysb = ysb_pool.tile([128, BLOC * V], F32, tag="ysb")
                nc.vector.tensor_scalar_add(ysb[:], py[:, 0:BLOC * V], pb_sb[:, mc:mc + 1])
                nc.sync.dma_start(
                    y_d.ap()[:, mc * 128:(mc + 1) * 128, :].rearrange(
                        "b p v -> p b v"),
                    ysb[:].rearrange("p (b v) -> p b v", b=BLOC),
                )

    nc.compile()
    return nc


def _get_program(nreps: int = 1):
    if nreps not in _PROG_CACHE:
        _PROG_CACHE[nreps] = _build_program(nreps)
    return _PROG_CACHE[nreps]


def _host_prep(x_cls, x_patch, q_w, k_w, v_w, temp, proj_w, proj_b):
    scale = 1.0 / math.sqrt(HD)
    tvec = np.repeat(temp.reshape(H).astype(np.float64), HD)
    q_ws = q_w.astype(np.float64) * (scale * tvec)[:, None]
    k64 = k_w.astype(np.float64)
    v64 = v_w.astype(np.float64)
    p64 = proj_w.astype(np.float64)
    g = np.empty((H, C, C), dtype=np.float64)
    w = np.empty((H, C, C), dtype=np.float64)
    for h in range(H):
        sl = slice(h * HD, (h + 1) * HD)
        g[h] = q_ws[sl, :].T @ k64[sl, :]          # [cin'(K), cin(M)]
        w[h] = (p64[:, sl] @ v64[sl, :]).T         # W_h.T = [cin(K), co(M)]
    g_bf = np.ascontiguousarray((g * ALPHA).astype(ml_dtypes.bfloat16))
    w_bf = np.ascontiguousarray(w.astype(ml_dtypes.bfloat16))
    pb = np.ascontiguousarray(proj_b.reshape(C, 1).astype(np.float32))
    return g_bf, w_bf, pb


def _make_in_maps(x_cls, x_patch, g_bf, w_bf, pb):
    xp_full = x_patch.reshape(B, C, KT)
    xpf = xp_full.astype(ml_dtypes.float8_e4m3)              # [B, C, KT]
    xpt = np.empty((B, KT, C1), dtype=ml_dtypes.bfloat16)    # [B, KT, C+1]
    xpt[:, :, 0:C] = xp_full.astype(ml_dtypes.bfloat16).transpose(0, 2, 1)
    xpt[:, :, C] = np.asarray(1.0, dtype=ml_dtypes.bfloat16)
    xc = np.ascontiguousarray(x_cls.reshape(B, C, V).astype(np.float32))
    in_maps = []
    for c in range(NCORES):
        bs = slice(c * BLOC, (c + 1) * BLOC)
        in_maps.append({
            "xc": xc[bs],
            "xpf": np.ascontiguousarray(xpf[bs]),
            "xpt": np.ascontiguousarray(xpt[bs]),
            "g": g_bf, "w": w_bf, "pb": pb,
        })
    return in_maps


def kernel(x_cls, x_patch, q_w, k_w, v_w, temp, proj_w, proj_b):
    g_bf, w_bf, pb = _host_prep(
        x_cls, x_patch, q_w, k_w, v_w, temp, proj_w, proj_b)
    nc = _get_program()
    in_maps = _make_in_maps(x_cls, x_patch, g_bf, w_bf, pb)
    res = bass_utils.run_bass_kernel_spmd(
        nc, in_maps, core_ids=list(range(NCORES)))
    out = np.concatenate([res.results[c]["y"] for c in range(NCORES)], axis=0)
    return out.reshape(B, C, 1, V).astype(np.float32)
